# revision 1
# baseline (speedup 1.0000x reference)
"""Multi-head attention (B=4, N=2048, H=1024, 16 heads) on 8 NeuronCores.

Sharding: core c -> (batch b = c//2, head-group g = c%2) with 8 heads per
group.  Each core computes QKV projection for its group, attention over its
8 heads, and a partial out-projection against its group's w_out columns.
The host sums the two partial products per batch and adds b_out.

All on-device layouts avoid transposes entirely:
  - host supplies x[b].T (plus a ones row so qkv biases ride the contraction)
  - QT/KT are produced directly in [head-dims, tokens] layout
  - scoresT = KT.T-slices @ QT gives P already transposed for the PV matmul
  - a ones column appended to V yields the softmax denominator in the same
    PSUM accumulation as PV (max-subtraction-free softmax: scores ~ N(0,1),
    safely inside fp32 exp range)
"""

import numpy as np

B, N, H, NH = 4, 2048, 1024, 16
HD = 64
G = 2            # head-groups = cores per batch
GH = NH // G     # heads per group
GF = GH * HD     # features per group (512)
NPAIR = GH // 2  # head pairs per group
HT = 9           # h-tiles incl. bias row
AUG = HT * 128   # 1152
NT = N // 128    # token tiles
VW = GH * 65     # v tile width incl. ones columns

DTYPE = "f32r"   # "f32r" | "bf16" compute dtype for matmul operands

_NC_CACHE = {}


class _Ctx:
    pass


def _make_ctx(nc, dtype, rep):
    import concourse.mybir as mybir

    c = _Ctx()
    c.nc = nc
    c.mybir = mybir
    c.CD = mybir.dt.float32r if dtype == "f32r" else mybir.dt.bfloat16
    c.F32 = mybir.dt.float32
    c.Exp = mybir.ActivationFunctionType.Exp
    c.R = f"r{rep}_"
    return c


def _phase1(c, tc, xt_d, wqkv_d, qkT, vt):
    """QKV projection: fills qkT (QT pairs 0-3, KT pairs 4-7) and vt."""
    nc, R, CD, F32 = c.nc, c.R, c.CD, c.F32
    with (
        tc.tile_pool(name=f"{R}wq_pool", bufs=1) as wq_pool,
        tc.tile_pool(name=f"{R}xt_pool", bufs=18) as xt_pool,
        tc.tile_pool(name=f"{R}p1_psum", bufs=4, space="PSUM") as p1_psum,
    ):
        wq = [
            wq_pool.tile([128, 2 * GF + VW], CD, name=f"{R}wq{i}") for i in range(HT)
        ]
        for qb in range(4):  # 512-token column blocks
            xt = [
                xt_pool.tile([128, 512], CD, tag="xt", name=f"{R}xt_{qb}_{i}")
                for i in range(HT)
            ]
            for i in range(HT):
                if qb == 0:
                    # interleave weight loads with the first activation block
                    # so the first accumulation chain starts ~1 MB in, not 8 MB
                    nc.sync.dma_start(wq[i][:], wqkv_d[i * 128 : (i + 1) * 128, :])
                nc.sync.dma_start(
                    xt[i][:],
                    xt_d[i * 128 : (i + 1) * 128, qb * 512 : (qb + 1) * 512],
                )
            # QT/KT row-tiles: out[feat_pair_dims, tokens]
            for rt in range(8):
                ps = p1_psum.tile([128, 512], F32, tag="p1ps", name=f"{R}p1q_{qb}_{rt}")
                for ht in range(HT):
                    nc.tensor.matmul(
                        ps[:],
                        wq[ht][:, rt * 128 : (rt + 1) * 128],
                        xt[ht][:],
                        start=(ht == 0),
                        stop=(ht == HT - 1),
                    )
                nc.vector.tensor_copy(qkT[rt][:, qb * 512 : (qb + 1) * 512], ps[:])
            # V token-tiles: out[tokens, vfeat interleaved with ones cols]
            for vtl in range(4):
                tt = qb * 4 + vtl
                ps = p1_psum.tile([128, VW], F32, tag="p1ps", name=f"{R}p1v_{qb}_{vtl}")
                for ht in range(HT):
                    nc.tensor.matmul(
                        ps[:, 0:512],
                        xt[ht][:, vtl * 128 : (vtl + 1) * 128],
                        wq[ht][:, 2 * GF : 2 * GF + 512],
                        start=(ht == 0),
                        stop=(ht == HT - 1),
                    )
                    nc.tensor.matmul(
                        ps[:, 512:VW],
                        xt[ht][:, vtl * 128 : (vtl + 1) * 128],
                        wq[ht][:, 2 * GF + 512 : 2 * GF + VW],
                        start=(ht == 0),
                        stop=(ht == HT - 1),
                    )
                nc.vector.tensor_copy(vt[tt][:], ps[:])


def _phase2(c, tc, qkT, vt, attnT, ones64):
    """Attention per head pair; writes normalized transposed output attnT."""
    nc, R, CD, F32, Exp = c.nc, c.R, c.CD, c.F32, c.Exp
    QB2 = 1024
    DEPTH = 2  # software-pipeline depth: PV of iter i emitted after exp(i+DEPTH)
    with (
        tc.tile_pool(name=f"{R}pt_pool", bufs=6) as pt_pool,
        tc.tile_pool(name=f"{R}norm_pool", bufs=2) as norm_pool,
        tc.tile_pool(name=f"{R}ps_pool", bufs=2, space="PSUM") as ps_pool,
        tc.tile_pool(name=f"{R}po_pool", bufs=2, space="PSUM") as po_pool,
    ):
        for qb in range(N // QB2):
            for p in range(NPAIR):
                po = [
                    po_pool.tile([65, QB2], F32, tag="po", name=f"{R}po_{p}_{qb}_{h}")
                    for h in range(2)
                ]

                def emit_pv(item):
                    pt, ikt, h = item
                    vslice = vt[ikt][:, (p * 2 + h) * 65 : (p * 2 + h + 1) * 65]
                    for hf in range(2):
                        nc.tensor.matmul(
                            po[h][:, hf * 512 : (hf + 1) * 512],
                            vslice,
                            pt[:, hf * 512 : (hf + 1) * 512],
                            start=(ikt == 0),
                            stop=(ikt == NT - 1),
                        )

                pending = []
                for ikt in range(NT):
                    for h in range(2):
                        ps = ps_pool.tile(
                            [128, QB2], F32, tag="ps", name=f"{R}ps_{p}_{qb}_{ikt}_{h}"
                        )
                        for hf in range(2):
                            nc.tensor.matmul(
                                ps[:, hf * 512 : (hf + 1) * 512],
                                qkT[NPAIR + p][
                                    h * 64 : (h + 1) * 64, ikt * 128 : (ikt + 1) * 128
                                ],
                                qkT[p][
                                    h * 64 : (h + 1) * 64,
                                    qb * QB2 + hf * 512 : qb * QB2 + (hf + 1) * 512,
                                ],
                                start=True,
                                stop=True,
                                tile_position=(h * 64, 0),
                            )
                        pt = pt_pool.tile(
                            [128, QB2], CD, tag="pt", name=f"{R}pt_{p}_{qb}_{ikt}_{h}"
                        )
                        nc.scalar.activation(
                            pt[:], ps[:], Exp, scale=float(HD) ** -0.5
                        )
                        pending.append((pt, ikt, h))
                        if len(pending) > DEPTH:
                            emit_pv(pending.pop(0))
                for item in pending:
                    emit_pv(item)
                for h in range(2):
                    # evacuate the accumulator to SBUF so its 2 PSUM banks
                    # free before the normalization chain completes
                    poc = norm_pool.tile(
                        [65, QB2], F32, tag="poc", name=f"{R}poc_{p}_{qb}_{h}"
                    )
                    nc.vector.tensor_copy(poc[:], po[h][:])
                    recip = norm_pool.tile(
                        [1, QB2], F32, tag="recip", name=f"{R}rc_{p}_{qb}_{h}"
                    )
                    nc.vector.reciprocal(recip[:], poc[64:65, :])
                    pb = ps_pool.tile(
                        [64, QB2], F32, tag="ps", name=f"{R}pb_{p}_{qb}_{h}"
                    )
                    for hf in range(2):
                        nc.tensor.matmul(
                            pb[:, hf * 512 : (hf + 1) * 512],
                            ones64[:],
                            recip[:, hf * 512 : (hf + 1) * 512],
                            start=True,
                            stop=True,
                        )
                    bcast = norm_pool.tile(
                        [64, QB2], F32, tag="bcast", name=f"{R}bc_{p}_{qb}_{h}"
                    )
                    nc.vector.tensor_copy(bcast[:], pb[:])
                    nc.vector.tensor_mul(
                        attnT[p][h * 64 : (h + 1) * 64, qb * QB2 : (qb + 1) * QB2],
                        poc[0:64, :],
                        bcast[:],
                    )


def _phase3(c, tc, attnT, wo_d, out_d):
    """Partial out-projection: out = attnT.T @ wo."""
    nc, R, CD, F32 = c.nc, c.R, c.CD, c.F32
    with (
        tc.tile_pool(name=f"{R}wo_pool", bufs=1) as wo_pool,
        tc.tile_pool(name=f"{R}out_pool", bufs=3) as out_pool,
        tc.tile_pool(name=f"{R}p3_psum", bufs=4, space="PSUM") as p3_psum,
    ):
        wo = [wo_pool.tile([128, H], CD, name=f"{R}wo{i}") for i in range(NPAIR)]
        for i in range(NPAIR):
            nc.sync.dma_start(wo[i][:], wo_d[i * 128 : (i + 1) * 128, :])
        for tt in range(NT):
            ob = out_pool.tile([128, H], F32, tag="ob", name=f"{R}ob{tt}")
            for nb in range(2):
                ps = p3_psum.tile([128, 512], F32, tag="p3", name=f"{R}p3_{tt}_{nb}")
                for jt in range(NPAIR):
                    nc.tensor.matmul(
                        ps[:],
                        attnT[jt][:, tt * 128 : (tt + 1) * 128],
                        wo[jt][:, nb * 512 : (nb + 1) * 512],
                        start=(jt == 0),
                        stop=(jt == NPAIR - 1),
                    )
                nc.vector.tensor_copy(ob[:, nb * 512 : (nb + 1) * 512], ps[:])
            nc.sync.dma_start(out_d[tt * 128 : (tt + 1) * 128, :], ob[:])


def _build_body(c, tc, xt_d, wqkv_d, wo_d, out_d, phases):
    nc, R, CD, F32 = c.nc, c.R, c.CD, c.F32
    with (
        tc.tile_pool(name=f"{R}qk_pool", bufs=1) as qk_pool,
        tc.tile_pool(name=f"{R}v_pool", bufs=1) as v_pool,
        tc.tile_pool(name=f"{R}const_pool", bufs=1) as const_pool,
    ):
        qkT = [qk_pool.tile([128, N], CD, name=f"{R}qkT{i}") for i in range(8)]
        vt = [v_pool.tile([128, VW], CD, name=f"{R}v{i}") for i in range(NT)]
        ones64 = const_pool.tile([1, 64], F32, name=f"{R}ones64")
        nc.vector.memset(ones64[:], 1.0)

        if 1 in phases:
            _phase1(c, tc, xt_d, wqkv_d, qkT, vt)
        with tc.tile_pool(name=f"{R}attnT_pool", bufs=1) as attnT_pool:
            attnT = [
                attnT_pool.tile([128, N], CD, name=f"{R}attnT{i}")
                for i in range(NPAIR)
            ]
            if 2 in phases:
                _phase2(c, tc, qkT, vt, attnT, ones64)
            if 3 in phases:
                _phase3(c, tc, attnT, wo_d, out_d)


def _build_nc(reps=1, dtype=None, phases=(1, 2, 3)):
    from concourse import bacc
    import concourse.mybir as mybir
    import concourse.tile as tile

    dtype = dtype or DTYPE
    CD = mybir.dt.float32r if dtype == "f32r" else mybir.dt.bfloat16
    F32 = mybir.dt.float32

    nc = bacc.Bacc("TRN2", target_bir_lowering=False)
    xt_d = nc.dram_tensor("xt", [AUG, N], CD, kind="ExternalInput")
    # columns: Q (GF) | K (GF) | V interleaved per head [64 weights | ones]
    wqkv_d = nc.dram_tensor("wqkv", [AUG, 2 * GF + VW], CD, kind="ExternalInput")
    wo_d = nc.dram_tensor("wo", [GF, H], CD, kind="ExternalInput")
    out_d = nc.dram_tensor("out", [N, H], F32, kind="ExternalOutput")

    with tile.TileContext(nc) as tc:
        for rep in range(reps):
            c = _make_ctx(nc, dtype, rep)
            _build_body(c, tc, xt_d, wqkv_d, wo_d, out_d, phases)
    nc.finalize()
    return nc


def _get_nc():
    key = ("nc", DTYPE)
    if key not in _NC_CACHE:
        _NC_CACHE[key] = _build_nc()
    return _NC_CACHE[key]


def _np_dtype():
    if DTYPE == "f32r":
        return np.float32
    import ml_dtypes

    return ml_dtypes.bfloat16


def _prep_inputs(x, w_qkv, b_qkv, w_out):
    """Build per-core host-side input maps."""
    nd = _np_dtype()
    x = np.asarray(x, dtype=np.float32)
    w_qkv = np.asarray(w_qkv, dtype=np.float32)
    b_qkv = np.asarray(b_qkv, dtype=np.float32)
    w_out = np.asarray(w_out, dtype=np.float32)

    wqkv_aug, wo_t = [], []
    for g in range(G):
        w = np.zeros((AUG, 2 * GF + VW), np.float32)
        for k in range(2):  # q, k blocks of w_qkv rows
            rows = slice(k * H + g * GF, k * H + (g + 1) * GF)
            w[:H, k * GF : (k + 1) * GF] = w_qkv[rows, :].T
            w[H, k * GF : (k + 1) * GF] = b_qkv[rows]
        for h in range(GH):  # v block, 65 cols per head
            rows = slice(2 * H + g * GF + h * HD, 2 * H + g * GF + (h + 1) * HD)
            col = 2 * GF + h * 65
            w[:H, col : col + HD] = w_qkv[rows, :].T
            w[H, col : col + HD] = b_qkv[rows]
            w[H, col + HD] = 1.0
        wqkv_aug.append(w.astype(nd))
        wo_t.append(
            np.ascontiguousarray(w_out[:, g * GF : (g + 1) * GF].T).astype(nd)
        )

    xts = []
    for b in range(B):
        xa = np.zeros((AUG, N), np.float32)
        xa[:H] = x[b].T
        xa[H] = 1.0
        xts.append(xa.astype(nd))

    in_maps = []
    for cc in range(B * G):
        b, g = divmod(cc, G)
        in_maps.append({"xt": xts[b], "wqkv": wqkv_aug[g], "wo": wo_t[g]})
    return in_maps


def run_sharded(x, w_qkv, b_qkv, w_out, b_out, trace=False):
    """Run the SPMD kernel; returns (out, BassKernelResults)."""
    from concourse.bass_utils import run_bass_kernel_spmd

    in_maps = _prep_inputs(x, w_qkv, b_qkv, w_out)
    nc = _get_nc()
    bkr = run_bass_kernel_spmd(nc, in_maps, list(range(B * G)), trace=trace)
    res = bkr.results
    b_out = np.asarray(b_out, dtype=np.float32)
    out = np.empty((B, N, H), np.float32)
    for b in range(B):
        out[b] = res[G * b]["out"] + res[G * b + 1]["out"] + b_out[None, :]
    return out, bkr


def kernel(x, w_qkv, b_qkv, w_out, b_out):
    out, _ = run_sharded(x, w_qkv, b_qkv, w_out, b_out)
    return out



# revision 2
# speedup vs baseline: 2.0355x; 2.0355x over previous
"""Multi-head attention (B=4, N=2048, H=1024, 16 heads) on 8 NeuronCores — v2.

Sharding: core c -> (batch b = c//2, head-group g = c%2), 8 heads per group.

Design (per core, bf16 compute):
  The softmax exp stream on the Activation engine (256 x [128,1024] ~= 267us)
  is the hard floor; all other work hides in its slack.
  - head: minimal Q/K projection for pair 0 only, fed by p-major consolidated
    DMAs so the first scores start ~10us in.
  - 8 attention windows (qb outer, head-pair inner), each an ACT-bound exp
    stream: scores [ktok, qtok] (h-outer) -> exp -> pt in SBUF. All other PE
    work runs as cost-budgeted filler between steps: V projection and the
    rest of K0 (window 0), later pairs' Q/K projections, per-(qt,h) PV chains
    (pt stationary, V moving, po [qtok,65] one PSUM bank each, 16-matmul
    accumulation), per-partition reciprocal+scalar-mul normalization,
    matmul-transpose groups back to attnT [feat, qtok], and the finished
    query block's out-projection.
  - PSUM: scores 3x[128,1024] (6 banks) + 2 rotating work banks; one
    accumulation group per 2KB bank.
  - tail: last window's PV drain interleaved with the final out-projection.
"""

import numpy as np

B, N, H, NH = 4, 2048, 1024, 16
HD = 64
G = 2            # head-groups = cores per batch
GH = NH // G     # 8 heads per group
GF = GH * HD     # 512 features per group
HT = 8           # contraction tiles (H/128)
NT = N // 128    # 16 token tiles
VW = GH * 65     # 520: v tile width incl. interleaved ones column per head
QB = 1024        # query block per attention window
NQT = QB // 128  # 8 query tiles per window
NPAIR = GH // 2  # 4 head pairs per group
# wqk DRAM column-block order: K(p) at 2p, Q(p) at 2p+1 — the head's K0|Q0
# blocks form one contiguous leading chunk
NEWCOL = {**{4 + p: 2 * p for p in range(4)}, **{p: 2 * p + 1 for p in range(4)}}

DTYPE = "bf16"

_NC_CACHE = {}


def _emit(nc, tc, R, CD, F32, Exp):
    from concourse.masks import make_identity

    scale = float(HD) ** -0.5

    work_ref = [None]
    with (
        tc.tile_pool(name=f"{R}const", bufs=1) as const_pool,
        tc.tile_pool(name=f"{R}w", bufs=1) as w_pool,
        tc.tile_pool(name=f"{R}qk", bufs=1) as qk_pool,
        tc.tile_pool(name=f"{R}v", bufs=1) as v_pool,
        tc.tile_pool(name=f"{R}attnT", bufs=1) as attnT_pool,
        tc.tile_pool(name=f"{R}attq", bufs=17) as attq_pool,
        tc.tile_pool(name=f"{R}rc", bufs=4) as rc_pool,
        tc.tile_pool(name=f"{R}ob", bufs=3) as ob_pool,
        tc.tile_pool(name=f"{R}work", bufs=2, space="PSUM") as work,
    ):
        work_ref[0] = work
        ident = const_pool.tile([128, 128], CD, name=f"{R}ident")
        bqk = const_pool.tile([128, 8], F32, name=f"{R}bqk")
        bv = const_pool.tile([128, GF], F32, name=f"{R}bv")
        warm = const_pool.tile([128, 2], F32, name=f"{R}warm")

        # p-major consolidated operand tensors: one SBUF tile per class,
        # loaded with a handful of large strided DMAs
        xtb = const_pool.tile([128, HT * N], CD, name=f"{R}xtb")
        wqkb = const_pool.tile([128, HT * 1024], CD, name=f"{R}wqkb")
        wvb = const_pool.tile([128, HT * GF], CD, name=f"{R}wvb")
        wob = const_pool.tile([128, NPAIR * H], CD, name=f"{R}wob")
        qkT = [qk_pool.tile([128, N], CD, name=f"{R}qkT{i}") for i in range(8)]
        vt = [v_pool.tile([128, VW], CD, name=f"{R}vt{i}") for i in range(NT)]
        attnT = [
            attnT_pool.tile([128, N], CD, name=f"{R}attnT{i}")
            for i in range(NPAIR)
        ]

        def xs(ht, a, b):
            return xtb[:, ht * N + a : ht * N + b]

        def wq(ht, a, b):
            return wqkb[:, ht * 1024 + a : ht * 1024 + b]

        def wv(ht):
            return wvb[:, ht * GF : (ht + 1) * GF]

        def wo(jt, a, b):
            return wob[:, jt * H + a : jt * H + b]

        def dma_xt(c):
            src = nc.t.xt[:, :].rearrange("p (t n) -> p t n", t=HT)
            dst = xtb[:].rearrange("p (t n) -> p t n", t=HT)
            nc.sync.dma_start(
                dst[:, :, c * 512 : (c + 1) * 512],
                src[:, :, c * 512 : (c + 1) * 512],
            )

        def dma_wqk(lo, hi):
            src = nc.t.wqk[:, :].rearrange("p (t n) -> p t n", t=HT)
            dst = wqkb[:].rearrange("p (t n) -> p t n", t=HT)
            nc.sync.dma_start(
                dst[:, :, lo * 128 : hi * 128], src[:, :, lo * 128 : hi * 128]
            )

        dma_wqk(0, 2)   # K0 | Q0
        dma_xt(0)
        dma_xt(1)
        nc.sync.dma_start(bqk[:], nc.t.bqk[:, :])
        nc.sync.dma_start(wvb[:], nc.t.wv[:, :])
        dma_xt(2)
        dma_xt(3)
        nc.sync.dma_start(bv[:], nc.t.bv[:, :])
        dma_wqk(2, 4)   # K1 | Q1
        dma_wqk(4, 8)
        nc.sync.dma_start(wob[:], nc.t.wo[:, :])

        make_identity(nc, ident[:])
        for t in range(NT):
            r = vt[t][:].rearrange("p (h w) -> p h w", h=GH, w=65)
            nc.gpsimd.memset(r[:, :, 64:65], 1.0)

        # warm the activation table (avoids a JIT table load before exp 0)
        nc.vector.memset(warm[:], 0.0)
        nc.scalar.activation(warm[:, 0:1], warm[:, 1:2], Exp, scale=1.0)

        # ---- fill emitters (micro-thunks with PE-cost tags) ---------------
        def qk_fill_parts(rt, c):
            """qkT[rt][:, c*512:(c+1)*512] = (x @ wqk_rt).T + bias."""
            cell = {}
            j = NEWCOL[rt]

            def part(k, cell=cell):
                if k == 0:
                    wp = work_ref[0]
                    cell["ps"] = wp.tile(
                        [128, 512], F32, tag="work", name=f"{R}qk_{rt}_{c}"
                    )
                ps = cell["ps"]
                for ht in range(k * 2, k * 2 + 2):
                    nc.tensor.matmul(
                        ps[:],
                        wq(ht, j * 128, (j + 1) * 128),
                        xs(ht, c * 512, (c + 1) * 512),
                        start=(ht == 0),
                        stop=(ht == HT - 1),
                    )
                if k == 3:
                    nc.vector.tensor_scalar_add(
                        qkT[rt][:, c * 512 : (c + 1) * 512],
                        ps[:],
                        bqk[:, j : j + 1],
                    )

            return [(430, lambda k=k: part(k)) for k in range(4)]

        def qk_fill(rt, c):
            for _, t in qk_fill_parts(rt, c):
                t()

        def v_fill_pair(tt, p):
            """vt[tt] pair-p V columns (2 heads, interleaved ones) + bias."""
            ps = work.tile([128, 128], F32, tag="work", name=f"{R}v_{tt}_{p}")
            for ht in range(HT):
                nc.tensor.matmul(
                    ps[:],
                    xs(ht, tt * 128, (tt + 1) * 128),
                    wvb[:, ht * GF + p * 128 : ht * GF + (p + 1) * 128],
                    start=(ht == 0),
                    stop=(ht == HT - 1),
                )
            vdst = vt[tt][:].rearrange(
                "p (h w) -> p h w", h=GH, w=65)[:, 2 * p : 2 * p + 2, 0:64]
            psr = ps[:].rearrange("p (h w) -> p h w", h=2, w=64)
            bvr = bv[:].rearrange(
                "p (h w) -> p h w", h=GH, w=64)[:, 2 * p : 2 * p + 2, :]
            nc.vector.tensor_add(vdst, psr, bvr)

        ob_tiles = {}

        def p3_fill(qb, tt, nb, pool=None, tag="work"):
            """out[tt rows, nb half] = sum_j attnT[j].T @ wo[j]; DMA on nb=1."""
            t = qb * NQT + tt
            pool = pool or work
            ps = pool.tile([128, 512], F32, tag=tag, name=f"{R}p3_{t}_{nb}")
            for jt in range(NPAIR):
                nc.tensor.matmul(
                    ps[:],
                    attnT[jt][:, t * 128 : (t + 1) * 128],
                    wo(jt, nb * 512, (nb + 1) * 512),
                    start=(jt == 0),
                    stop=(jt == NPAIR - 1),
                )
            if nb == 0:
                ob_tiles[t] = ob_pool.tile(
                    [128, H], F32, tag="ob", name=f"{R}ob{t}"
                )
            ob = ob_tiles[t]
            nc.vector.tensor_copy(ob[:, nb * 512 : (nb + 1) * 512], ps[:])
            if nb == 1:
                nc.sync.dma_start(nc.t.out[t * 128 : (t + 1) * 128, :], ob[:])

        # ---- head: just enough projection for the first scores ------------
        with tc.tile_pool(name=f"{R}head", bufs=2, space="PSUM") as hp:
            _saved = work_ref[0]
            work_ref[0] = hp
            # keep the PE continuously busy through the input-DMA window so
            # it reaches full p-state before the first projection fills
            wps = hp.tile([128, 128], F32, tag="warmps", name=f"{R}wps")
            for i in range(56):
                nc.tensor.matmul(wps[:], ident[:], ident[:],
                                 start=True, stop=True)
            qk_fill(4, 0)  # K pair 0, first key chunk
            qk_fill(0, 0)  # Q pair 0, qb0 columns
            qk_fill(0, 1)
            work_ref[0] = _saved

        # ---- attention windows --------------------------------------------
        fifo = []
        credit = [0.0]

        def consume(rate, cap=600.0):
            credit[0] = min(credit[0] + rate, cap)
            while fifo and credit[0] >= fifo[0][0]:
                cost, thunk = fifo.pop(0)
                thunk()
                credit[0] -= cost

        def chain(qt, h, p, qb, pts, aqs, pool=None, tag="work"):
            head = p * 2 + h
            pool = pool or work
            w = pool.tile([128, 512], F32, tag=tag,
                          name=f"{R}ch_{qb}_{p}_{qt}_{h}")
            for ikt in range(NT):
                nc.tensor.matmul(
                    w[:, 0:65],
                    pts[(ikt, h)][:, qt * 128 : (qt + 1) * 128],
                    vt[ikt][:, head * 65 : (head + 1) * 65],
                    start=(ikt == 0),
                    stop=(ikt == NT - 1),
                )
            rc = rc_pool.tile([128, 1], F32, tag="rc",
                              name=f"{R}rc_{qb}_{p}_{qt}_{h}")
            nc.vector.reciprocal(rc[:], w[:, 64:65])
            nc.vector.tensor_scalar_mul(
                aqs[qt][:, h * 64 : (h + 1) * 64], w[:, 0:64], rc[:]
            )

        def tgroup(q4, p, qb, aqs, pool=None, tag="work"):
            pool = pool or work
            w = pool.tile([128, 512], F32, tag=tag, name=f"{R}tg_{qb}_{p}_{q4}")
            for qi in range(4):
                nc.tensor.matmul(
                    w[:, qi * 128 : (qi + 1) * 128],
                    aqs[q4 * 4 + qi][:],
                    ident[:],
                    start=(qi == 0),
                    stop=(qi == 3),
                )
            nc.vector.tensor_copy(
                attnT[p][:, qb * QB + q4 * 512 : qb * QB + (q4 + 1) * 512],
                w[:],
            )

        last_items = []
        last_chains = {}
        last_aqs = []
        with tc.tile_pool(name=f"{R}pt", bufs=26) as pt_pool:
            with tc.tile_pool(name=f"{R}ps", bufs=3, space="PSUM") as ps_pool:
                for qb in range(N // QB):
                    for p in range(NPAIR):
                        last = qb == 1 and p == NPAIR - 1
                        # queue projection fills needed by later windows
                        if qb == 0:
                            if p == 0:
                                for c in (1, 2, 3):  # rest of K pair 0 (JIT)
                                    fifo.extend(qk_fill_parts(4, c))
                            for tt in range(NT):  # this pair's V tiles
                                fifo.append((470,
                                    lambda tt=tt, p=p: v_fill_pair(tt, p)))
                            if p < NPAIR - 1:
                                for c in range(4):
                                    fifo.extend(qk_fill_parts(5 + p, c))
                                for c in range(2):
                                    fifo.extend(qk_fill_parts(p + 1, c))
                            else:
                                for c in (2, 3):
                                    fifo.extend(qk_fill_parts(0, c))
                        elif p < NPAIR - 1:
                            for c in (2, 3):
                                fifo.extend(qk_fill_parts(p + 1, c))

                        pts = {}
                        attqs = [
                            attq_pool.tile(
                                [128, 128], CD,
                                tag="aqlast" if last else "attq",
                                bufs=8 if last else None,
                                name=f"{R}aq_{qb}_{p}_{qt}")
                            for qt in range(NQT)
                        ]
                        budget = 900 if (qb == 0 and p == 0) else (800 if qb == 0 else 550)
                        for h in range(2):
                            for ikt in range(NT):
                                ps = ps_pool.tile(
                                    [128, QB], F32, tag="ps",
                                    name=f"{R}ps_{qb}_{p}_{ikt}_{h}",
                                )
                                for hf in range(2):
                                    nc.tensor.matmul(
                                        ps[:, hf * 512 : (hf + 1) * 512],
                                        qkT[NPAIR + p][
                                            h * 64 : (h + 1) * 64,
                                            ikt * 128 : (ikt + 1) * 128,
                                        ],
                                        qkT[p][
                                            h * 64 : (h + 1) * 64,
                                            qb * QB + hf * 512 : qb * QB
                                            + (hf + 1) * 512,
                                        ],
                                        start=True,
                                        stop=True,
                                        tile_position=(h * 64, 0),
                                    )
                                pt = pt_pool.tile(
                                    [128, QB], CD, tag="pt",
                                    name=f"{R}pt_{qb}_{p}_{ikt}_{h}",
                                )
                                nc.scalar.activation(
                                    pt[:], ps[:], Exp, scale=scale
                                )
                                pts[(ikt, h)] = pt
                                consume(budget)
                            # h-phase end: queue this half's PV chains
                            for qt in range(NQT):
                                if last and h == 1:
                                    last_chains[qt] = (
                                        lambda qt=qt, h=h, p=p, qb=qb,
                                        pts=pts, aqs=attqs, **kw: chain(
                                            qt, h, p, qb, pts, aqs, **kw))
                                else:
                                    fifo.append((440,
                                        lambda qt=qt, h=h, p=p, qb=qb,
                                        pts=pts, aqs=attqs: chain(
                                            qt, h, p, qb, pts, aqs)))
                        n_end = 0
                        for q4 in range(2):
                            if last:
                                last_aqs = attqs
                            else:
                                fifo.append((450,
                                    lambda q4=q4, p=p, qb=qb, aqs=attqs:
                                    tgroup(q4, p, qb, aqs)))
                                n_end += 1
                        if qb == 1 and p in (1, 2, 3):
                            lo, hi = (p - 1) * 3, min((p - 1) * 3 + 3, NQT)
                            for tt in range(lo, hi):
                                for nb in range(2):
                                    fifo.append((900,
                                        lambda tt=tt, nb=nb: p3_fill(0, tt,
                                                                     nb)))
                                    n_end += 1
                        if not last:
                            # drain carryover: its chains must be emitted
                            # before the next window's pt buffers rotate onto
                            # their inputs (deadlock prevention)
                            while len(fifo) > n_end:
                                fifo.pop(0)[1]()
            # ---- tail: drain last window interleaved with out-projection --
            with tc.tile_pool(name=f"{R}tail", bufs=4, space="PSUM") as tp:
                while fifo:
                    fifo.pop(0)[1]()
                for half in range(2):
                    for qt in range(half * 4, half * 4 + 4):
                        last_chains[qt](pool=tp, tag="tps")
                    tgroup(half, NPAIR - 1, 1, last_aqs, pool=tp, tag="tps")
                    for tt in range(half * 4, half * 4 + 4):
                        for nb in range(2):
                            p3_fill(1, tt, nb, pool=tp, tag="tps")


class _T:
    pass


def _build_nc(reps=1, dtype=None, phases=None):
    from concourse import bacc
    import concourse.mybir as mybir
    import concourse.tile as tile

    dtype = dtype or DTYPE
    CD = mybir.dt.float32r if dtype == "f32r" else mybir.dt.bfloat16
    F32 = mybir.dt.float32
    Exp = mybir.ActivationFunctionType.Exp

    nc = bacc.Bacc("TRN2", target_bir_lowering=False)
    t = _T()
    t.xt = nc.dram_tensor("xt", [128, HT * N], CD, kind="ExternalInput")
    t.wqk = nc.dram_tensor("wqk", [128, HT * 1024], CD, kind="ExternalInput")
    t.wv = nc.dram_tensor("wv", [128, HT * GF], CD, kind="ExternalInput")
    t.bqk = nc.dram_tensor("bqk", [128, 8], F32, kind="ExternalInput")
    t.bv = nc.dram_tensor("bv", [128, GF], F32, kind="ExternalInput")
    t.wo = nc.dram_tensor("wo", [128, NPAIR * H], CD, kind="ExternalInput")
    t.out = nc.dram_tensor("out", [N, H], F32, kind="ExternalOutput")
    nc.t = t

    with tile.TileContext(nc) as tc:
        for rep in range(reps):
            _emit(nc, tc, f"r{rep}_", CD, F32, Exp)
    nc.finalize()
    return nc


def _get_nc():
    key = ("nc", DTYPE)
    if key not in _NC_CACHE:
        _NC_CACHE[key] = _build_nc()
    return _NC_CACHE[key]


def _np_dtype():
    if DTYPE == "f32r":
        return np.float32
    import ml_dtypes

    return ml_dtypes.bfloat16


def _pmajor(a, tiles):
    """[tiles*128, W] -> [128, tiles*W] with tile index as the middle axis."""
    w = a.shape[1]
    return np.ascontiguousarray(
        a.reshape(tiles, 128, w).transpose(1, 0, 2).reshape(128, tiles * w)
    )


def _prep_inputs(x, w_qkv, b_qkv, w_out):
    """Build per-core host-side input maps."""
    nd = _np_dtype()
    x = np.asarray(x, dtype=np.float32)
    w_qkv = np.asarray(w_qkv, dtype=np.float32)
    b_qkv = np.asarray(b_qkv, dtype=np.float32)
    w_out = np.asarray(w_out, dtype=np.float32)

    per_g = []
    for g in range(G):
        qs = slice(g * GF, (g + 1) * GF)
        ks = slice(H + g * GF, H + (g + 1) * GF)
        vs = slice(2 * H + g * GF, 2 * H + (g + 1) * GF)
        qkblocks = [None] * 8
        for p in range(4):
            qkblocks[2 * p] = w_qkv[ks, :][p * 128 : (p + 1) * 128, :].T
            qkblocks[2 * p + 1] = w_qkv[qs, :][p * 128 : (p + 1) * 128, :].T
        wqk = np.concatenate(qkblocks, axis=1)  # [H, 1024], K0|Q0|K1|Q1|...
        wv = np.ascontiguousarray(w_qkv[vs, :].T)  # [H, 512]
        bqk = np.zeros((128, 8), np.float32)
        for p in range(4):
            bqk[:, 2 * p] = b_qkv[ks][p * 128 : (p + 1) * 128]
            bqk[:, 2 * p + 1] = b_qkv[qs][p * 128 : (p + 1) * 128]
        bv = np.broadcast_to(b_qkv[vs], (128, GF)).copy()
        wo = np.ascontiguousarray(w_out[:, g * GF : (g + 1) * GF].T)
        per_g.append(
            {
                "wqk": _pmajor(wqk, HT).astype(nd),
                "wv": _pmajor(wv, HT).astype(nd),
                "bqk": bqk,
                "bv": bv,
                "wo": _pmajor(wo, NPAIR).astype(nd),
            }
        )

    xts = [
        _pmajor(np.ascontiguousarray(x[b].T), HT).astype(nd) for b in range(B)
    ]

    in_maps = []
    for cc in range(B * G):
        b, g = divmod(cc, G)
        in_maps.append({"xt": xts[b], **per_g[g]})
    return in_maps


def run_sharded(x, w_qkv, b_qkv, w_out, b_out, trace=False):
    """Run the SPMD kernel; returns (out, BassKernelResults)."""
    from concourse.bass_utils import run_bass_kernel_spmd

    in_maps = _prep_inputs(x, w_qkv, b_qkv, w_out)
    nc = _get_nc()
    bkr = run_bass_kernel_spmd(nc, in_maps, list(range(B * G)), trace=trace)
    res = bkr.results
    b_out = np.asarray(b_out, dtype=np.float32)
    out = np.empty((B, N, H), np.float32)
    for b in range(B):
        out[b] = (
            res[G * b]["out"].astype(np.float32)
            + res[G * b + 1]["out"].astype(np.float32)
            + b_out[None, :]
        )
    return out, bkr


def kernel(x, w_qkv, b_qkv, w_out, b_out):
    out, _ = run_sharded(x, w_qkv, b_qkv, w_out, b_out)
    return out


# revision 3
# speedup vs baseline: 2.2211x; 1.0912x over previous
"""Multi-head attention (B=4, N=2048, H=1024, 16 heads) on 8 NeuronCores — v2.

Sharding: core c -> (batch b = c//2, head-group g = c%2), 8 heads per group.

Design (per core, bf16 compute):
  The softmax exp stream on the Activation engine (256 x [128,1024] ~= 267us)
  is the hard floor; all other work hides in its slack.
  - head: minimal Q/K projection for pair 0 only, fed by p-major consolidated
    DMAs so the first scores start ~10us in.
  - 8 attention windows (qb outer, head-pair inner), each an ACT-bound exp
    stream: scores [ktok, qtok] (h-outer) -> exp -> pt in SBUF. All other PE
    work runs as cost-budgeted filler between steps: V projection and the
    rest of K0 (window 0), later pairs' Q/K projections, per-(qt,h) PV chains
    (pt stationary, V moving, po [qtok,65] one PSUM bank each, 16-matmul
    accumulation), per-partition reciprocal+scalar-mul normalization,
    matmul-transpose groups back to attnT [feat, qtok], and the finished
    query block's out-projection.
  - PSUM: scores 3x[128,1024] (6 banks) + 2 rotating work banks; one
    accumulation group per 2KB bank.
  - tail: last window's PV drain interleaved with the final out-projection.
"""

import numpy as np

B, N, H, NH = 4, 2048, 1024, 16
HD = 64
G = 2            # head-groups = cores per batch
GH = NH // G     # 8 heads per group
GF = GH * HD     # 512 features per group
HT = 8           # contraction tiles (H/128)
NT = N // 128    # 16 token tiles
VW = GH * 65     # 520: v tile width incl. interleaved ones column per head
QB = 1024        # query block per attention window
NQT = QB // 128  # 8 query tiles per window
NPAIR = GH // 2  # 4 head pairs per group
# wqk DRAM column-block order: K(p) at 2p, Q(p) at 2p+1 — the head's K0|Q0
# blocks form one contiguous leading chunk
NEWCOL = {**{4 + p: 2 * p for p in range(4)}, **{p: 2 * p + 1 for p in range(4)}}

DTYPE = "bf16"

_NC_CACHE = {}


def _emit(nc, tc, R, CD, F32, Exp):
    from concourse.masks import make_identity

    scale = float(HD) ** -0.5

    work_ref = [None]
    with (
        tc.tile_pool(name=f"{R}const", bufs=1) as const_pool,
        tc.tile_pool(name=f"{R}w", bufs=1) as w_pool,
        tc.tile_pool(name=f"{R}qk", bufs=1) as qk_pool,
        tc.tile_pool(name=f"{R}v", bufs=1) as v_pool,
        tc.tile_pool(name=f"{R}attnT", bufs=1) as attnT_pool,
        tc.tile_pool(name=f"{R}attq", bufs=17) as attq_pool,
        tc.tile_pool(name=f"{R}rc", bufs=4) as rc_pool,
        tc.tile_pool(name=f"{R}ob", bufs=3) as ob_pool,
        tc.tile_pool(name=f"{R}work", bufs=2, space="PSUM") as work,
    ):
        work_ref[0] = work
        ident = const_pool.tile([128, 128], CD, name=f"{R}ident")
        bqk = const_pool.tile([128, 8], F32, name=f"{R}bqk")
        bv = const_pool.tile([128, GF], F32, name=f"{R}bv")
        warm = const_pool.tile([128, 2], F32, name=f"{R}warm")

        # p-major consolidated operand tensors: one SBUF tile per class,
        # loaded with a handful of large strided DMAs
        xtb = const_pool.tile([128, HT * N], CD, name=f"{R}xtb")
        wqkb = const_pool.tile([128, HT * 1024], CD, name=f"{R}wqkb")
        wvb = const_pool.tile([128, HT * GF], CD, name=f"{R}wvb")
        wob = const_pool.tile([128, NPAIR * H], CD, name=f"{R}wob")
        qkT = [qk_pool.tile([128, N], CD, name=f"{R}qkT{i}") for i in range(8)]
        vt = [v_pool.tile([128, VW], CD, name=f"{R}vt{i}") for i in range(NT)]
        attnT = [
            attnT_pool.tile([128, N], CD, name=f"{R}attnT{i}")
            for i in range(NPAIR)
        ]

        def xs(ht, a, b):
            return xtb[:, ht * N + a : ht * N + b]

        def wq(ht, a, b):
            return wqkb[:, ht * 1024 + a : ht * 1024 + b]

        def wv(ht):
            return wvb[:, ht * GF : (ht + 1) * GF]

        def wo(jt, a, b):
            return wob[:, jt * H + a : jt * H + b]

        def dma_xt(c, eng=None):
            src = nc.t.xt[:, :].rearrange("p (t n) -> p t n", t=HT)
            dst = xtb[:].rearrange("p (t n) -> p t n", t=HT)
            (eng or nc.sync).dma_start(
                dst[:, :, c * 512 : (c + 1) * 512],
                src[:, :, c * 512 : (c + 1) * 512],
            )

        def dma_wqk(lo, hi):
            src = nc.t.wqk[:, :].rearrange("p (t n) -> p t n", t=HT)
            dst = wqkb[:].rearrange("p (t n) -> p t n", t=HT)
            nc.sync.dma_start(
                dst[:, :, lo * 128 : hi * 128], src[:, :, lo * 128 : hi * 128]
            )

        dma_wqk(0, 2)   # K0 | Q0
        dma_xt(0)
        dma_xt(1)
        nc.sync.dma_start(bqk[:], nc.t.bqk[:, :])
        nc.sync.dma_start(wvb[:], nc.t.wv[:, :])
        dma_xt(2)
        dma_xt(3)
        nc.sync.dma_start(bv[:], nc.t.bv[:, :])
        dma_wqk(2, 4)   # K1 | Q1
        dma_wqk(4, 8)
        nc.sync.dma_start(wob[:], nc.t.wo[:, :])

        make_identity(nc, ident[:])
        for t in range(NT):
            r = vt[t][:].rearrange("p (h w) -> p h w", h=GH, w=65)
            nc.gpsimd.memset(r[:, :, 64:65], 1.0)

        # warm the activation table (avoids a JIT table load before exp 0)
        nc.vector.memset(warm[:], 0.0)
        nc.scalar.activation(warm[:, 0:1], warm[:, 1:2], Exp, scale=1.0)

        # ---- fill emitters (micro-thunks with PE-cost tags) ---------------
        def qk_fill_parts(rt, c):
            """qkT[rt][:, c*512:(c+1)*512] = (x @ wqk_rt).T + bias."""
            cell = {}
            j = NEWCOL[rt]

            def part(k, cell=cell):
                if k == 0:
                    wp = work_ref[0]
                    cell["ps"] = wp.tile(
                        [128, 512], F32, tag="work", name=f"{R}qk_{rt}_{c}"
                    )
                ps = cell["ps"]
                for ht in range(k * 2, k * 2 + 2):
                    nc.tensor.matmul(
                        ps[:],
                        wq(ht, j * 128, (j + 1) * 128),
                        xs(ht, c * 512, (c + 1) * 512),
                        start=(ht == 0),
                        stop=(ht == HT - 1),
                    )
                if k == 3:
                    nc.vector.tensor_scalar_add(
                        qkT[rt][:, c * 512 : (c + 1) * 512],
                        ps[:],
                        bqk[:, j : j + 1],
                    )

            return [(430, lambda k=k: part(k)) for k in range(4)]

        def qk_fill(rt, c):
            for _, t in qk_fill_parts(rt, c):
                t()

        def v_fill_pair(tt, p):
            """vt[tt] pair-p V columns (2 heads, interleaved ones) + bias."""
            ps = work.tile([128, 128], F32, tag="work", name=f"{R}v_{tt}_{p}")
            for ht in range(HT):
                nc.tensor.matmul(
                    ps[:],
                    xs(ht, tt * 128, (tt + 1) * 128),
                    wvb[:, ht * GF + p * 128 : ht * GF + (p + 1) * 128],
                    start=(ht == 0),
                    stop=(ht == HT - 1),
                )
            vdst = vt[tt][:].rearrange(
                "p (h w) -> p h w", h=GH, w=65)[:, 2 * p : 2 * p + 2, 0:64]
            psr = ps[:].rearrange("p (h w) -> p h w", h=2, w=64)
            bvr = bv[:].rearrange(
                "p (h w) -> p h w", h=GH, w=64)[:, 2 * p : 2 * p + 2, :]
            nc.vector.tensor_add(vdst, psr, bvr)

        ob_tiles = {}

        def p3_parts(qb, tt, nb, pool=None, tag="work"):
            """out[tt rows, nb half] = sum_j attnT[j].T @ wo[j]; DMA on nb=1."""
            t = qb * NQT + tt
            pool = pool or work
            cell = {}

            def part(k, cell=cell):
                if k == 0:
                    cell["ps"] = pool.tile(
                        [128, 512], F32, tag=tag, name=f"{R}p3_{t}_{nb}"
                    )
                ps = cell["ps"]
                for jt in range(k * 2, k * 2 + 2):
                    nc.tensor.matmul(
                        ps[:],
                        attnT[jt][:, t * 128 : (t + 1) * 128],
                        wo(jt, nb * 512, (nb + 1) * 512),
                        start=(jt == 0),
                        stop=(jt == NPAIR - 1),
                    )
                if k == 1:
                    if nb == 0:
                        ob_tiles[t] = ob_pool.tile(
                            [128, H], F32, tag="ob", name=f"{R}ob{t}"
                        )
                    ob = ob_tiles[t]
                    nc.vector.tensor_copy(
                        ob[:, nb * 512 : (nb + 1) * 512], ps[:]
                    )
                    if nb == 1:
                        nc.sync.dma_start(
                            nc.t.out[t * 128 : (t + 1) * 128, :], ob[:]
                        )

            return [(440, lambda k=k: part(k)) for k in range(2)]

        def p3_fill(qb, tt, nb, pool=None, tag="work"):
            for _, th in p3_parts(qb, tt, nb, pool=pool, tag=tag):
                th()

        # ---- head: just enough projection for the first scores ------------
        with tc.tile_pool(name=f"{R}head", bufs=2, space="PSUM") as hp:
            _saved = work_ref[0]
            work_ref[0] = hp
            # keep the PE continuously busy through the input-DMA window so
            # it reaches full p-state before the first projection fills
            wps = hp.tile([128, 128], F32, tag="warmps", name=f"{R}wps")
            for i in range(56):
                nc.tensor.matmul(wps[:], ident[:], ident[:],
                                 start=True, stop=True)
            qk_fill(4, 0)  # K pair 0, first key chunk
            qk_fill(0, 0)  # Q pair 0, qb0 columns
            qk_fill(0, 1)
            work_ref[0] = _saved

        # ---- attention windows --------------------------------------------
        fifo = []
        credit = [0.0]

        def consume(rate, cap=600.0):
            credit[0] = min(credit[0] + rate, cap)
            while fifo and credit[0] >= fifo[0][0]:
                cost, thunk = fifo.pop(0)
                thunk()
                credit[0] -= cost

        def chain(qt, h, p, qb, pts, aqs, pool=None, tag="work"):
            head = p * 2 + h
            pool = pool or work
            w = pool.tile([128, 512], F32, tag=tag,
                          name=f"{R}ch_{qb}_{p}_{qt}_{h}")
            for ikt in range(NT):
                nc.tensor.matmul(
                    w[:, 0:65],
                    pts[(ikt, h)][:, qt * 128 : (qt + 1) * 128],
                    vt[ikt][:, head * 65 : (head + 1) * 65],
                    start=(ikt == 0),
                    stop=(ikt == NT - 1),
                )
            rc = rc_pool.tile([128, 1], F32, tag="rc",
                              name=f"{R}rc_{qb}_{p}_{qt}_{h}")
            nc.vector.reciprocal(rc[:], w[:, 64:65])
            nc.vector.tensor_scalar_mul(
                aqs[qt][:, h * 64 : (h + 1) * 64], w[:, 0:64], rc[:]
            )

        def tgroup(q4, p, qb, aqs, pool=None, tag="work"):
            pool = pool or work
            w = pool.tile([128, 512], F32, tag=tag, name=f"{R}tg_{qb}_{p}_{q4}")
            for qi in range(4):
                nc.tensor.matmul(
                    w[:, qi * 128 : (qi + 1) * 128],
                    aqs[q4 * 4 + qi][:],
                    ident[:],
                    start=(qi == 0),
                    stop=(qi == 3),
                )
            nc.vector.tensor_copy(
                attnT[p][:, qb * QB + q4 * 512 : qb * QB + (q4 + 1) * 512],
                w[:],
            )

        last_items = []
        last_chains = {}
        last_aqs = []
        with tc.tile_pool(name=f"{R}pt", bufs=26) as pt_pool:
            with tc.tile_pool(name=f"{R}ps", bufs=3, space="PSUM") as ps_pool:
                for qb in range(N // QB):
                    for p in range(NPAIR):
                        last = qb == 1 and p == NPAIR - 1
                        # queue projection fills needed by later windows
                        if qb == 0:
                            if p == 0:
                                for c in (1, 2, 3):  # rest of K pair 0 (JIT)
                                    fifo.extend(qk_fill_parts(4, c))
                            for tt in range(NT):  # this pair's V tiles
                                fifo.append((470,
                                    lambda tt=tt, p=p: v_fill_pair(tt, p)))
                            if p < NPAIR - 1:
                                for c in range(4):
                                    fifo.extend(qk_fill_parts(5 + p, c))
                                for c in range(2):
                                    fifo.extend(qk_fill_parts(p + 1, c))
                            else:
                                for c in (2, 3):
                                    fifo.extend(qk_fill_parts(0, c))
                        elif p < NPAIR - 1:
                            for c in (2, 3):
                                fifo.extend(qk_fill_parts(p + 1, c))

                        pts = {}
                        attqs = [
                            attq_pool.tile(
                                [128, 128], CD,
                                tag="aqlast" if last else "attq",
                                bufs=8 if last else None,
                                name=f"{R}aq_{qb}_{p}_{qt}")
                            for qt in range(NQT)
                        ]
                        budget = 900 if (qb == 0 and p == 0) else (800 if qb == 0 else 550)
                        for h in range(2):
                            for ikt in range(NT):
                                ps = ps_pool.tile(
                                    [128, QB], F32, tag="ps",
                                    name=f"{R}ps_{qb}_{p}_{ikt}_{h}",
                                )
                                for hf in range(2):
                                    nc.tensor.matmul(
                                        ps[:, hf * 512 : (hf + 1) * 512],
                                        qkT[NPAIR + p][
                                            h * 64 : (h + 1) * 64,
                                            ikt * 128 : (ikt + 1) * 128,
                                        ],
                                        qkT[p][
                                            h * 64 : (h + 1) * 64,
                                            qb * QB + hf * 512 : qb * QB
                                            + (hf + 1) * 512,
                                        ],
                                        start=True,
                                        stop=True,
                                        tile_position=(h * 64, 0),
                                    )
                                pt = pt_pool.tile(
                                    [128, QB], CD, tag="pt",
                                    name=f"{R}pt_{qb}_{p}_{ikt}_{h}",
                                )
                                nc.scalar.activation(
                                    pt[:], ps[:], Exp, scale=scale
                                )
                                pts[(ikt, h)] = pt
                                consume(budget)
                            # h-phase end: queue this half's PV chains
                            for qt in range(NQT):
                                if last and h == 1:
                                    last_chains[qt] = (
                                        lambda qt=qt, h=h, p=p, qb=qb,
                                        pts=pts, aqs=attqs, **kw: chain(
                                            qt, h, p, qb, pts, aqs, **kw))
                                else:
                                    fifo.append((440,
                                        lambda qt=qt, h=h, p=p, qb=qb,
                                        pts=pts, aqs=attqs: chain(
                                            qt, h, p, qb, pts, aqs)))
                        n_end = 0
                        for q4 in range(2):
                            if last:
                                last_aqs = attqs
                            else:
                                fifo.append((450,
                                    lambda q4=q4, p=p, qb=qb, aqs=attqs:
                                    tgroup(q4, p, qb, aqs)))
                                n_end += 1
                        if qb == 1 and p in (1, 2, 3):
                            lo, hi = (p - 1) * 3, min((p - 1) * 3 + 3, NQT)
                            for tt in range(lo, hi):
                                for nb in range(2):
                                    fifo.append((900,
                                        lambda tt=tt, nb=nb: p3_fill(0, tt,
                                                                     nb)))
                                    n_end += 1
                        if not last:
                            # drain carryover: its chains must be emitted
                            # before the next window's pt buffers rotate onto
                            # their inputs (deadlock prevention)
                            while len(fifo) > n_end:
                                fifo.pop(0)[1]()
            # ---- tail: drain last window interleaved with out-projection --
            with tc.tile_pool(name=f"{R}tail", bufs=4, space="PSUM") as tp:
                while fifo:
                    fifo.pop(0)[1]()
                for half in range(2):
                    for qt in range(half * 4, half * 4 + 4):
                        if qt % 2:
                            last_chains[qt](pool=work, tag="work")
                        else:
                            last_chains[qt](pool=tp, tag="tps")
                    tgroup(half, NPAIR - 1, 1, last_aqs, pool=work, tag="work")
                    for tt in range(half * 4, half * 4 + 4):
                        for nb in range(2):
                            p3_fill(1, tt, nb, pool=tp, tag="tps")


class _T:
    pass


def _build_nc(reps=1, dtype=None, phases=None):
    from concourse import bacc
    import concourse.mybir as mybir
    import concourse.tile as tile

    dtype = dtype or DTYPE
    CD = mybir.dt.float32r if dtype == "f32r" else mybir.dt.bfloat16
    F32 = mybir.dt.float32
    Exp = mybir.ActivationFunctionType.Exp

    nc = bacc.Bacc("TRN2", target_bir_lowering=False)
    t = _T()
    t.xt = nc.dram_tensor("xt", [128, HT * N], CD, kind="ExternalInput")
    t.wqk = nc.dram_tensor("wqk", [128, HT * 1024], CD, kind="ExternalInput")
    t.wv = nc.dram_tensor("wv", [128, HT * GF], CD, kind="ExternalInput")
    t.bqk = nc.dram_tensor("bqk", [128, 8], F32, kind="ExternalInput")
    t.bv = nc.dram_tensor("bv", [128, GF], F32, kind="ExternalInput")
    t.wo = nc.dram_tensor("wo", [128, NPAIR * H], CD, kind="ExternalInput")
    t.out = nc.dram_tensor("out", [N, H], F32, kind="ExternalOutput")
    nc.t = t

    with tile.TileContext(nc) as tc:
        for rep in range(reps):
            _emit(nc, tc, f"r{rep}_", CD, F32, Exp)
    nc.finalize()
    return nc


def _get_nc():
    key = ("nc", DTYPE)
    if key not in _NC_CACHE:
        _NC_CACHE[key] = _build_nc()
    return _NC_CACHE[key]


def _np_dtype():
    if DTYPE == "f32r":
        return np.float32
    import ml_dtypes

    return ml_dtypes.bfloat16


def _pmajor(a, tiles):
    """[tiles*128, W] -> [128, tiles*W] with tile index as the middle axis."""
    w = a.shape[1]
    return np.ascontiguousarray(
        a.reshape(tiles, 128, w).transpose(1, 0, 2).reshape(128, tiles * w)
    )


def _prep_inputs(x, w_qkv, b_qkv, w_out):
    """Build per-core host-side input maps."""
    nd = _np_dtype()
    x = np.asarray(x, dtype=np.float32)
    w_qkv = np.asarray(w_qkv, dtype=np.float32)
    b_qkv = np.asarray(b_qkv, dtype=np.float32)
    w_out = np.asarray(w_out, dtype=np.float32)

    per_g = []
    for g in range(G):
        qs = slice(g * GF, (g + 1) * GF)
        ks = slice(H + g * GF, H + (g + 1) * GF)
        vs = slice(2 * H + g * GF, 2 * H + (g + 1) * GF)
        qkblocks = [None] * 8
        for p in range(4):
            qkblocks[2 * p] = w_qkv[ks, :][p * 128 : (p + 1) * 128, :].T
            qkblocks[2 * p + 1] = w_qkv[qs, :][p * 128 : (p + 1) * 128, :].T
        wqk = np.concatenate(qkblocks, axis=1)  # [H, 1024], K0|Q0|K1|Q1|...
        wv = np.ascontiguousarray(w_qkv[vs, :].T)  # [H, 512]
        bqk = np.zeros((128, 8), np.float32)
        for p in range(4):
            bqk[:, 2 * p] = b_qkv[ks][p * 128 : (p + 1) * 128]
            bqk[:, 2 * p + 1] = b_qkv[qs][p * 128 : (p + 1) * 128]
        bv = np.broadcast_to(b_qkv[vs], (128, GF)).copy()
        wo = np.ascontiguousarray(w_out[:, g * GF : (g + 1) * GF].T)
        per_g.append(
            {
                "wqk": _pmajor(wqk, HT).astype(nd),
                "wv": _pmajor(wv, HT).astype(nd),
                "bqk": bqk,
                "bv": bv,
                "wo": _pmajor(wo, NPAIR).astype(nd),
            }
        )

    xts = [
        _pmajor(np.ascontiguousarray(x[b].T), HT).astype(nd) for b in range(B)
    ]

    in_maps = []
    for cc in range(B * G):
        b, g = divmod(cc, G)
        in_maps.append({"xt": xts[b], **per_g[g]})
    return in_maps


def run_sharded(x, w_qkv, b_qkv, w_out, b_out, trace=False):
    """Run the SPMD kernel; returns (out, BassKernelResults)."""
    from concourse.bass_utils import run_bass_kernel_spmd

    in_maps = _prep_inputs(x, w_qkv, b_qkv, w_out)
    nc = _get_nc()
    bkr = run_bass_kernel_spmd(nc, in_maps, list(range(B * G)), trace=trace)
    res = bkr.results
    b_out = np.asarray(b_out, dtype=np.float32)
    out = np.empty((B, N, H), np.float32)
    for b in range(B):
        out[b] = (
            res[G * b]["out"].astype(np.float32)
            + res[G * b + 1]["out"].astype(np.float32)
            + b_out[None, :]
        )
    return out, bkr


def kernel(x, w_qkv, b_qkv, w_out, b_out):
    out, _ = run_sharded(x, w_qkv, b_qkv, w_out, b_out)
    return out


# revision 4
# speedup vs baseline: 2.2387x; 1.0079x over previous
"""Multi-head attention (B=4, N=2048, H=1024, 16 heads) on 8 NeuronCores — v2.

Sharding: core c -> (batch b = c//2, head-group g = c%2), 8 heads per group.

Design (per core, bf16 compute):
  The softmax exp stream on the Activation engine (256 x [128,1024] ~= 267us)
  is the hard floor; all other work hides in its slack.
  - head: minimal Q/K projection for pair 0 only, fed by p-major consolidated
    DMAs so the first scores start ~10us in.
  - 8 attention windows (qb outer, head-pair inner), each an ACT-bound exp
    stream: scores [ktok, qtok] (h-outer) -> exp -> pt in SBUF. All other PE
    work runs as cost-budgeted filler between steps: V projection and the
    rest of K0 (window 0), later pairs' Q/K projections, per-(qt,h) PV chains
    (pt stationary, V moving, po [qtok,65] one PSUM bank each, 16-matmul
    accumulation), per-partition reciprocal+scalar-mul normalization,
    matmul-transpose groups back to attnT [feat, qtok], and the finished
    query block's out-projection.
  - PSUM: scores 3x[128,1024] (6 banks) + 2 rotating work banks; one
    accumulation group per 2KB bank.
  - tail: last window's PV drain interleaved with the final out-projection.
"""

import numpy as np

B, N, H, NH = 4, 2048, 1024, 16
HD = 64
G = 2            # head-groups = cores per batch
GH = NH // G     # 8 heads per group
GF = GH * HD     # 512 features per group
HT = 8           # contraction tiles (H/128)
NT = N // 128    # 16 token tiles
VW = GH * 65     # 520: v tile width incl. interleaved ones column per head
QB = 1024        # query block per attention window
NQT = QB // 128  # 8 query tiles per window
NPAIR = GH // 2  # 4 head pairs per group
# wqk DRAM column-block order: K(p) at 2p, Q(p) at 2p+1 — the head's K0|Q0
# blocks form one contiguous leading chunk
NEWCOL = {**{4 + p: 2 * p for p in range(4)}, **{p: 2 * p + 1 for p in range(4)}}

DTYPE = "bf16"

_NC_CACHE = {}


def _emit(nc, tc, R, CD, F32, Exp):
    from concourse.masks import make_identity

    scale = float(HD) ** -0.5

    work_ref = [None]
    with (
        tc.tile_pool(name=f"{R}const", bufs=1) as const_pool,
        tc.tile_pool(name=f"{R}w", bufs=1) as w_pool,
        tc.tile_pool(name=f"{R}qk", bufs=1) as qk_pool,
        tc.tile_pool(name=f"{R}v", bufs=1) as v_pool,
        tc.tile_pool(name=f"{R}attnT", bufs=1) as attnT_pool,
        tc.tile_pool(name=f"{R}attq", bufs=17) as attq_pool,
        tc.tile_pool(name=f"{R}rc", bufs=4) as rc_pool,
        tc.tile_pool(name=f"{R}ob", bufs=2) as ob_pool,
        tc.tile_pool(name=f"{R}work", bufs=2, space="PSUM") as work,
    ):
        work_ref[0] = work
        ident = const_pool.tile([128, 128], CD, name=f"{R}ident")
        bqk = const_pool.tile([128, 8], F32, name=f"{R}bqk")
        bv = const_pool.tile([128, GF], F32, name=f"{R}bv")
        warm = const_pool.tile([128, 2], F32, name=f"{R}warm")

        # p-major consolidated operand tensors: one SBUF tile per class,
        # loaded with a handful of large strided DMAs
        xtb = const_pool.tile([128, HT * N], CD, name=f"{R}xtb")
        wqkb = const_pool.tile([128, HT * 1024], CD, name=f"{R}wqkb")
        wvb = const_pool.tile([128, HT * GF], CD, name=f"{R}wvb")
        wob = const_pool.tile([128, NPAIR * H], CD, name=f"{R}wob")
        qkT = [qk_pool.tile([128, N], CD, name=f"{R}qkT{i}") for i in range(8)]
        vt = [v_pool.tile([128, VW], CD, name=f"{R}vt{i}") for i in range(NT)]
        attnT = [
            attnT_pool.tile([128, N], CD, name=f"{R}attnT{i}")
            for i in range(NPAIR)
        ]

        def xs(ht, a, b):
            return xtb[:, ht * N + a : ht * N + b]

        def wq(ht, a, b):
            return wqkb[:, ht * 1024 + a : ht * 1024 + b]

        def wv(ht):
            return wvb[:, ht * GF : (ht + 1) * GF]

        def wo(jt, a, b):
            return wob[:, jt * H + a : jt * H + b]

        def dma_xt(c, eng=None):
            src = nc.t.xt[:, :].rearrange("p (t n) -> p t n", t=HT)
            dst = xtb[:].rearrange("p (t n) -> p t n", t=HT)
            (eng or nc.sync).dma_start(
                dst[:, :, c * 512 : (c + 1) * 512],
                src[:, :, c * 512 : (c + 1) * 512],
            )

        def dma_wqk(lo, hi):
            src = nc.t.wqk[:, :].rearrange("p (t n) -> p t n", t=HT)
            dst = wqkb[:].rearrange("p (t n) -> p t n", t=HT)
            nc.sync.dma_start(
                dst[:, :, lo * 128 : hi * 128], src[:, :, lo * 128 : hi * 128]
            )

        dma_wqk(0, 2)   # K0 | Q0
        dma_xt(0)
        dma_xt(1)
        nc.sync.dma_start(bqk[:], nc.t.bqk[:, :])
        nc.sync.dma_start(wvb[:], nc.t.wv[:, :])
        dma_xt(2)
        dma_xt(3)
        nc.sync.dma_start(bv[:], nc.t.bv[:, :])
        dma_wqk(2, 4)   # K1 | Q1
        dma_wqk(4, 8)
        nc.sync.dma_start(wob[:], nc.t.wo[:, :])

        make_identity(nc, ident[:])
        for t in range(NT):
            r = vt[t][:].rearrange("p (h w) -> p h w", h=GH, w=65)
            nc.gpsimd.memset(r[:, :, 64:65], 1.0)

        # warm the activation table (avoids a JIT table load before exp 0)
        nc.vector.memset(warm[:], 0.0)
        nc.scalar.activation(warm[:, 0:1], warm[:, 1:2], Exp, scale=1.0)

        # ---- fill emitters (micro-thunks with PE-cost tags) ---------------
        def qk_fill_parts(rt, c):
            """qkT[rt][:, c*512:(c+1)*512] = (x @ wqk_rt).T + bias."""
            cell = {}
            j = NEWCOL[rt]

            def part(k, cell=cell):
                if k == 0:
                    wp = work_ref[0]
                    cell["ps"] = wp.tile(
                        [128, 512], F32, tag="work", name=f"{R}qk_{rt}_{c}"
                    )
                ps = cell["ps"]
                for ht in range(k * 2, k * 2 + 2):
                    nc.tensor.matmul(
                        ps[:],
                        wq(ht, j * 128, (j + 1) * 128),
                        xs(ht, c * 512, (c + 1) * 512),
                        start=(ht == 0),
                        stop=(ht == HT - 1),
                    )
                if k == 3:
                    nc.vector.tensor_scalar_add(
                        qkT[rt][:, c * 512 : (c + 1) * 512],
                        ps[:],
                        bqk[:, j : j + 1],
                    )

            return [(430, lambda k=k: part(k)) for k in range(4)]

        def qk_fill(rt, c):
            for _, t in qk_fill_parts(rt, c):
                t()

        def v_fill_pair(tt, p):
            """vt[tt] pair-p V columns (2 heads, interleaved ones) + bias."""
            ps = work.tile([128, 128], F32, tag="work", name=f"{R}v_{tt}_{p}")
            for ht in range(HT):
                nc.tensor.matmul(
                    ps[:],
                    xs(ht, tt * 128, (tt + 1) * 128),
                    wvb[:, ht * GF + p * 128 : ht * GF + (p + 1) * 128],
                    start=(ht == 0),
                    stop=(ht == HT - 1),
                )
            vdst = vt[tt][:].rearrange(
                "p (h w) -> p h w", h=GH, w=65)[:, 2 * p : 2 * p + 2, 0:64]
            psr = ps[:].rearrange("p (h w) -> p h w", h=2, w=64)
            bvr = bv[:].rearrange(
                "p (h w) -> p h w", h=GH, w=64)[:, 2 * p : 2 * p + 2, :]
            nc.vector.tensor_add(vdst, psr, bvr)

        ob_tiles = {}

        def p3_parts(qb, tt, nb, pool=None, tag="work"):
            """out[tt rows, nb half] = sum_j attnT[j].T @ wo[j]; DMA on nb=1."""
            t = qb * NQT + tt
            pool = pool or work
            cell = {}

            def part(k, cell=cell):
                if k == 0:
                    cell["ps"] = pool.tile(
                        [128, 512], F32, tag=tag, name=f"{R}p3_{t}_{nb}"
                    )
                ps = cell["ps"]
                for jt in range(k * 2, k * 2 + 2):
                    nc.tensor.matmul(
                        ps[:],
                        attnT[jt][:, t * 128 : (t + 1) * 128],
                        wo(jt, nb * 512, (nb + 1) * 512),
                        start=(jt == 0),
                        stop=(jt == NPAIR - 1),
                    )
                if k == 1:
                    if nb == 0:
                        ob_tiles[t] = ob_pool.tile(
                            [128, H], F32, tag="ob", name=f"{R}ob{t}"
                        )
                    ob = ob_tiles[t]
                    nc.vector.tensor_copy(
                        ob[:, nb * 512 : (nb + 1) * 512], ps[:]
                    )
                    if nb == 1:
                        nc.sync.dma_start(
                            nc.t.out[t * 128 : (t + 1) * 128, :], ob[:]
                        )

            return [(440, lambda k=k: part(k)) for k in range(2)]

        def p3_fill(qb, tt, nb, pool=None, tag="work"):
            for _, th in p3_parts(qb, tt, nb, pool=pool, tag=tag):
                th()

        # ---- head: just enough projection for the first scores ------------
        with tc.tile_pool(name=f"{R}head", bufs=2, space="PSUM") as hp:
            _saved = work_ref[0]
            work_ref[0] = hp
            # keep the PE continuously busy through the input-DMA window so
            # it reaches full p-state before the first projection fills
            wps = hp.tile([128, 128], F32, tag="warmps", name=f"{R}wps")
            for i in range(56):
                nc.tensor.matmul(wps[:], ident[:], ident[:],
                                 start=True, stop=True)
            qk_fill(4, 0)  # K pair 0, first key chunk
            qk_fill(0, 0)  # Q pair 0, qb0 columns
            qk_fill(0, 1)
            work_ref[0] = _saved

        # ---- attention windows --------------------------------------------
        fifo = []
        credit = [0.0]

        def consume(rate, cap=600.0):
            credit[0] = min(credit[0] + rate, cap)
            while fifo and credit[0] >= fifo[0][0]:
                cost, thunk = fifo.pop(0)
                thunk()
                credit[0] -= cost

        def chain(qt, h, p, qb, pts, aqs, pool=None, tag="work"):
            head = p * 2 + h
            pool = pool or work
            w = pool.tile([128, 512], F32, tag=tag,
                          name=f"{R}ch_{qb}_{p}_{qt}_{h}")
            for ikt in range(NT):
                nc.tensor.matmul(
                    w[:, 0:65],
                    pts[(ikt, h)][:, qt * 128 : (qt + 1) * 128],
                    vt[ikt][:, head * 65 : (head + 1) * 65],
                    start=(ikt == 0),
                    stop=(ikt == NT - 1),
                )
            rc = rc_pool.tile([128, 1], F32, tag="rc",
                              name=f"{R}rc_{qb}_{p}_{qt}_{h}")
            nc.vector.reciprocal(rc[:], w[:, 64:65])
            nc.vector.tensor_scalar_mul(
                aqs[qt][:, h * 64 : (h + 1) * 64], w[:, 0:64], rc[:]
            )

        def tgroup(q4, p, qb, aqs, pool=None, tag="work"):
            pool = pool or work
            w = pool.tile([128, 512], F32, tag=tag, name=f"{R}tg_{qb}_{p}_{q4}")
            for qi in range(4):
                nc.tensor.matmul(
                    w[:, qi * 128 : (qi + 1) * 128],
                    aqs[q4 * 4 + qi][:],
                    ident[:],
                    start=(qi == 0),
                    stop=(qi == 3),
                )
            nc.vector.tensor_copy(
                attnT[p][:, qb * QB + q4 * 512 : qb * QB + (q4 + 1) * 512],
                w[:],
            )

        last_items = []
        last_chains = {}
        last_aqs = []
        with tc.tile_pool(name=f"{R}pt", bufs=30) as pt_pool:
            with tc.tile_pool(name=f"{R}ps", bufs=3, space="PSUM") as ps_pool:
                for qb in range(N // QB):
                    for p in range(NPAIR):
                        last = qb == 1 and p == NPAIR - 1
                        # queue projection fills needed by later windows
                        if qb == 0:
                            if p == 0:
                                for c in (1, 2, 3):  # rest of K pair 0 (JIT)
                                    fifo.extend(qk_fill_parts(4, c))
                            for tt in range(NT):  # this pair's V tiles
                                fifo.append((470,
                                    lambda tt=tt, p=p: v_fill_pair(tt, p)))
                            if p < NPAIR - 1:
                                for c in range(4):
                                    fifo.extend(qk_fill_parts(5 + p, c))
                                for c in range(2):
                                    fifo.extend(qk_fill_parts(p + 1, c))
                            else:
                                for c in (2, 3):
                                    fifo.extend(qk_fill_parts(0, c))
                        elif p < NPAIR - 1:
                            for c in (2, 3):
                                fifo.extend(qk_fill_parts(p + 1, c))

                        pts = {}
                        attqs = [
                            attq_pool.tile(
                                [128, 128], CD,
                                tag="aqlast" if last else "attq",
                                bufs=8 if last else None,
                                name=f"{R}aq_{qb}_{p}_{qt}")
                            for qt in range(NQT)
                        ]
                        budget = 900 if (qb == 0 and p == 0) else (800 if qb == 0 else 550)
                        for h in range(2):
                            for ikt in range(NT):
                                ps = ps_pool.tile(
                                    [128, QB], F32, tag="ps",
                                    name=f"{R}ps_{qb}_{p}_{ikt}_{h}",
                                )
                                for hf in range(2):
                                    nc.tensor.matmul(
                                        ps[:, hf * 512 : (hf + 1) * 512],
                                        qkT[NPAIR + p][
                                            h * 64 : (h + 1) * 64,
                                            ikt * 128 : (ikt + 1) * 128,
                                        ],
                                        qkT[p][
                                            h * 64 : (h + 1) * 64,
                                            qb * QB + hf * 512 : qb * QB
                                            + (hf + 1) * 512,
                                        ],
                                        start=True,
                                        stop=True,
                                        tile_position=(h * 64, 0),
                                    )
                                pt = pt_pool.tile(
                                    [128, QB], CD, tag="pt",
                                    name=f"{R}pt_{qb}_{p}_{ikt}_{h}",
                                )
                                nc.scalar.activation(
                                    pt[:], ps[:], Exp, scale=scale
                                )
                                pts[(ikt, h)] = pt
                                consume(budget)
                            # h-phase end: queue this half's PV chains
                            for qt in range(NQT):
                                if last and h == 1:
                                    last_chains[qt] = (
                                        lambda qt=qt, h=h, p=p, qb=qb,
                                        pts=pts, aqs=attqs, **kw: chain(
                                            qt, h, p, qb, pts, aqs, **kw))
                                else:
                                    fifo.append((440,
                                        lambda qt=qt, h=h, p=p, qb=qb,
                                        pts=pts, aqs=attqs: chain(
                                            qt, h, p, qb, pts, aqs)))
                        n_end = 0
                        for q4 in range(2):
                            if last:
                                last_aqs = attqs
                            else:
                                fifo.append((450,
                                    lambda q4=q4, p=p, qb=qb, aqs=attqs:
                                    tgroup(q4, p, qb, aqs)))
                                n_end += 1
                        if qb == 1 and p in (1, 2, 3):
                            lo, hi = (p - 1) * 3, min((p - 1) * 3 + 3, NQT)
                            for tt in range(lo, hi):
                                for nb in range(2):
                                    fifo.append((900,
                                        lambda tt=tt, nb=nb: p3_fill(0, tt,
                                                                     nb)))
                                    n_end += 1
                        if not last:
                            # drain carryover: its chains must be emitted
                            # before the next window's pt buffers rotate onto
                            # their inputs (deadlock prevention)
                            while len(fifo) > n_end:
                                fifo.pop(0)[1]()
            # ---- tail: drain last window interleaved with out-projection --
            with tc.tile_pool(name=f"{R}tail", bufs=4, space="PSUM") as tp:
                while fifo:
                    fifo.pop(0)[1]()
                for half in range(2):
                    for qt in range(half * 4, half * 4 + 4):
                        if qt % 2:
                            last_chains[qt](pool=work, tag="work")
                        else:
                            last_chains[qt](pool=tp, tag="tps")
                    tgroup(half, NPAIR - 1, 1, last_aqs, pool=work, tag="work")
                    for tt in range(half * 4, half * 4 + 4):
                        for nb in range(2):
                            p3_fill(1, tt, nb, pool=tp, tag="tps")


class _T:
    pass


def _build_nc(reps=1, dtype=None, phases=None):
    from concourse import bacc
    import concourse.mybir as mybir
    import concourse.tile as tile

    dtype = dtype or DTYPE
    CD = mybir.dt.float32r if dtype == "f32r" else mybir.dt.bfloat16
    F32 = mybir.dt.float32
    Exp = mybir.ActivationFunctionType.Exp

    nc = bacc.Bacc("TRN2", target_bir_lowering=False)
    t = _T()
    t.xt = nc.dram_tensor("xt", [128, HT * N], CD, kind="ExternalInput")
    t.wqk = nc.dram_tensor("wqk", [128, HT * 1024], CD, kind="ExternalInput")
    t.wv = nc.dram_tensor("wv", [128, HT * GF], CD, kind="ExternalInput")
    t.bqk = nc.dram_tensor("bqk", [128, 8], F32, kind="ExternalInput")
    t.bv = nc.dram_tensor("bv", [128, GF], F32, kind="ExternalInput")
    t.wo = nc.dram_tensor("wo", [128, NPAIR * H], CD, kind="ExternalInput")
    t.out = nc.dram_tensor("out", [N, H], F32, kind="ExternalOutput")
    nc.t = t

    with tile.TileContext(nc) as tc:
        for rep in range(reps):
            _emit(nc, tc, f"r{rep}_", CD, F32, Exp)
    nc.finalize()
    return nc


def _get_nc():
    key = ("nc", DTYPE)
    if key not in _NC_CACHE:
        _NC_CACHE[key] = _build_nc()
    return _NC_CACHE[key]


def _np_dtype():
    if DTYPE == "f32r":
        return np.float32
    import ml_dtypes

    return ml_dtypes.bfloat16


def _pmajor(a, tiles):
    """[tiles*128, W] -> [128, tiles*W] with tile index as the middle axis."""
    w = a.shape[1]
    return np.ascontiguousarray(
        a.reshape(tiles, 128, w).transpose(1, 0, 2).reshape(128, tiles * w)
    )


def _prep_inputs(x, w_qkv, b_qkv, w_out):
    """Build per-core host-side input maps."""
    nd = _np_dtype()
    x = np.asarray(x, dtype=np.float32)
    w_qkv = np.asarray(w_qkv, dtype=np.float32)
    b_qkv = np.asarray(b_qkv, dtype=np.float32)
    w_out = np.asarray(w_out, dtype=np.float32)

    per_g = []
    for g in range(G):
        qs = slice(g * GF, (g + 1) * GF)
        ks = slice(H + g * GF, H + (g + 1) * GF)
        vs = slice(2 * H + g * GF, 2 * H + (g + 1) * GF)
        qkblocks = [None] * 8
        for p in range(4):
            qkblocks[2 * p] = w_qkv[ks, :][p * 128 : (p + 1) * 128, :].T
            qkblocks[2 * p + 1] = w_qkv[qs, :][p * 128 : (p + 1) * 128, :].T
        wqk = np.concatenate(qkblocks, axis=1)  # [H, 1024], K0|Q0|K1|Q1|...
        wv = np.ascontiguousarray(w_qkv[vs, :].T)  # [H, 512]
        bqk = np.zeros((128, 8), np.float32)
        for p in range(4):
            bqk[:, 2 * p] = b_qkv[ks][p * 128 : (p + 1) * 128]
            bqk[:, 2 * p + 1] = b_qkv[qs][p * 128 : (p + 1) * 128]
        bv = np.broadcast_to(b_qkv[vs], (128, GF)).copy()
        wo = np.ascontiguousarray(w_out[:, g * GF : (g + 1) * GF].T)
        per_g.append(
            {
                "wqk": _pmajor(wqk, HT).astype(nd),
                "wv": _pmajor(wv, HT).astype(nd),
                "bqk": bqk,
                "bv": bv,
                "wo": _pmajor(wo, NPAIR).astype(nd),
            }
        )

    xts = [
        _pmajor(np.ascontiguousarray(x[b].T), HT).astype(nd) for b in range(B)
    ]

    in_maps = []
    for cc in range(B * G):
        b, g = divmod(cc, G)
        in_maps.append({"xt": xts[b], **per_g[g]})
    return in_maps


def run_sharded(x, w_qkv, b_qkv, w_out, b_out, trace=False):
    """Run the SPMD kernel; returns (out, BassKernelResults)."""
    from concourse.bass_utils import run_bass_kernel_spmd

    in_maps = _prep_inputs(x, w_qkv, b_qkv, w_out)
    nc = _get_nc()
    bkr = run_bass_kernel_spmd(nc, in_maps, list(range(B * G)), trace=trace)
    res = bkr.results
    b_out = np.asarray(b_out, dtype=np.float32)
    out = np.empty((B, N, H), np.float32)
    for b in range(B):
        out[b] = (
            res[G * b]["out"].astype(np.float32)
            + res[G * b + 1]["out"].astype(np.float32)
            + b_out[None, :]
        )
    return out, bkr


def kernel(x, w_qkv, b_qkv, w_out, b_out):
    out, _ = run_sharded(x, w_qkv, b_qkv, w_out, b_out)
    return out
